# revision 1
# baseline (speedup 1.0000x reference)
"""Trainium2 Bass kernel for AttnPainterOil-style top-K stroke compositing.

Problem semantics (per pixel, fully independent):
  draw[n] = (n+1) * (alpha[n] > 0.1); top-K=10 of draw over N=256 strokes
  (descending) == the 10 highest-index strokes with alpha > 0.1 (for the
  target input distribution every pixel has >= 10 passing strokes, checked
  on the host below).  Gather alpha/color at those indices and composite
  back-to-front over a white canvas.

Streaming formulation used on device (front-to-back, strokes in descending
index order): maintain per-pixel transmittance T (init 1), accepted count k
(init 0) and color accumulator C (init 0).  For each stroke:
  ae = a * 1{a > 0.1} * 1{k < 10}
  k += 1{ae > 0}
  ta = ae * T ;  T -= ta ;  C += ta * c
Final canvas = C + T (white background).

Only the top D=30 strokes can ever enter any pixel's top-10 (the host
verifies >= 10 passing within the top D per pixel before using the device
path; anything else falls back to an exact host replication).  This cuts
device traffic 8.5x versus streaming all 256 strokes.

Sharding: pure data parallel, one batch element per NeuronCore (B=8, 8
cores).  Engine split: the whole serial per-stroke chain and the color
products run on DVE (GpSimd shares SBUF ports with DVE and co-running
them degrades both); PE accumulates the weighted colors into PSUM via
identity matmuls; a final DVE op adds the white background straight out
of PSUM.
"""

import numpy as np

B, N, W, K = 8, 256, 128, 10
ALPHA_THRESH = 0.1
D = 30          # strokes processed from the top (must cover every pixel's top-10)
P = 128         # partitions (pixel rows)
F = 128         # free dim (pixel cols)
G = 8           # strokes per color-DMA chunk
NCORES = 8

_nc_cache = {}


def _build_nc(depth):
    import concourse.bass as bass  # noqa: F401
    import concourse.tile as tile
    from concourse import bacc, mybir
    from concourse.vector_clock import ScopedClock

    op = mybir.AluOpType
    f32 = mybir.dt.float32

    class _OneShotTileContext(tile.TileContext):
        """TileContext with a slim exit: the drain alone (it waits on the
        global clock, including output-DMA completion) — no all-engine
        barriers and no per-semaphore clears.  Safe because every
        run_bass_kernel_spmd call builds and loads a fresh executable, so
        semaphore state never carries across runs."""

        def _drain_and_barrier(self, tick_clock, wait_clock):
            drain_inst = self.nc.sync.drain()
            wait_clock.add_sem_waits(
                drain_inst.ins, ScopedClock({None: tick_clock.global_clock})
            )
            popped = self.nc._tile_sem_poison_stack.pop()
            assert popped is self._sem_poison

    nc = bacc.Bacc("TRN2", target_bir_lowering=False, debug=False)

    alpha_d = nc.dram_tensor("alpha_in", [P, depth * F], f32, kind="ExternalInput").ap()
    color_d = nc.dram_tensor("color_in", [P, depth * 3 * F], f32, kind="ExternalInput").ap()
    ident_d = nc.dram_tensor("ident_in", [P, P], f32, kind="ExternalInput").ap()
    out_d = nc.dram_tensor("out", [P, 3 * F], f32, kind="ExternalOutput").ap()

    with _OneShotTileContext(nc) as tc:
        with (
            tc.tile_pool(name="const", bufs=1) as constp,
            tc.tile_pool(name="state", bufs=1) as statep,
            tc.tile_pool(name="alpha", bufs=2) as alphap,
            tc.tile_pool(name="ae0", bufs=2) as ae0p,
            tc.tile_pool(name="cpair", bufs=4) as cpairp,
            tc.tile_pool(name="cchunk", bufs=2) as cchunkp,
            tc.tile_pool(name="tap", bufs=2) as tap,
            tc.tile_pool(name="aep", bufs=2) as aep,
            tc.tile_pool(name="prodp", bufs=4) as prodp,
            tc.tile_pool(name="psum", bufs=1, space="PSUM") as psump,
        ):
            # ident via SWDGE (gpsimd queue) so it doesn't delay the HWDGE
            # input stream; it's only needed by the first matmul.
            ident = constp.tile([P, P], f32)
            nc.gpsimd.dma_start(ident[:], ident_d)

            kcnt = statep.tile([P, F], f32)
            T = statep.tile([P, F], f32)
            nc.vector.memset(kcnt[:], 0.0)
            nc.gpsimd.memset(T[:], 1.0)

            cacc = psump.tile([P, 3 * F], f32)

            # small first chunk: the opening compute waits on 128KB of
            # alpha instead of 256KB, and the first ae0 op is 2x shorter
            sizes = [4] + [G] * ((depth - 4) // G)
            rem = depth - sum(sizes)
            if rem:
                sizes.append(rem)
            chunks = []
            off = 0
            for g_sz in sizes:
                chunks.append((off, g_sz))
                off += g_sz

            def chain_ops(ss, ae0_s, ta_out):
                """Serial per-stroke mask/count/transmittance ops (all DVE)."""
                if ss < K:
                    ae = ae0_s          # gate reads k_{ss-1} <= ss <= 9 < 10: always open
                else:
                    ae_t = aep.tile([P, F], f32, tag="ae")
                    nc.vector.scalar_tensor_tensor(
                        ae_t[:], kcnt[:], 9.5, ae0_s, op0=op.is_lt, op1=op.mult
                    )
                    ae = ae_t[:]
                if ss < depth - 1:
                    nc.vector.scalar_tensor_tensor(
                        kcnt[:], ae, 0.0, kcnt[:], op0=op.is_gt, op1=op.add
                    )
                nc.vector.tensor_tensor(ta_out, ae, T[:], op=op.mult)
                nc.vector.tensor_tensor(T[:], T[:], ta_out, op=op.subtract)

            # Everything on DVE: GpSimd shares SBUF ports with DVE and
            # co-running them degrades DVE ~5x.  PE (own xbus ports)
            # accumulates the weighted colors without contention.
            for off, g_sz in chunks:
                # alpha per chunk: first compute waits on 256KB, not the
                # whole slab
                atile = alphap.tile([P, G * F], f32, tag="alpha")
                nc.sync.dma_start(
                    atile[:, : g_sz * F], alpha_d[:, off * F : (off + g_sz) * F]
                )

                # chunk 0: color in stroke-pair slices so the first product
                # isn't gated on a big transfer; later chunks: one DMA each
                first = off == 0
                if first:
                    ctiles = []
                    for s2 in range(g_sz // 2):
                        cpair = cpairp.tile([P, 2, 3, F], f32, tag="cpair")
                        lo = (off + 2 * s2) * 3 * F
                        c_src = color_d[:, lo : lo + 2 * 3 * F]
                        nc.sync.dma_start(
                            cpair[:], c_src.rearrange("p (s c f) -> p s c f", s=2, c=3)
                        )
                        ctiles.append(cpair)
                else:
                    cchunk = cchunkp.tile([P, G, 3, F], f32, tag="cchunk")
                    lo = off * 3 * F
                    c_src = color_d[:, lo : lo + g_sz * 3 * F]
                    nc.sync.dma_start(
                        cchunk[:, :g_sz],
                        c_src.rearrange("p (s c f) -> p s c f", s=g_sz, c=3),
                    )

                # ae0 = a * 1{a > thresh} for the whole chunk (batched)
                ae0 = ae0p.tile([P, G * F], f32, tag="ae0")
                a_sl = atile[:, : g_sz * F]
                nc.vector.scalar_tensor_tensor(
                    ae0[:, : g_sz * F], a_sl, ALPHA_THRESH, a_sl,
                    op0=op.is_gt, op1=op.mult,
                )

                # stroke pairs throughout: keeps PE uniformly busy (quads
                # idle PE between bursts and trigger HAM downclock)
                bs = 2
                s = 0
                while s < g_sz:
                    b = min(bs, g_sz - s)
                    ta_grp = tap.tile([P, 2, F], f32, tag="ta")
                    for j in range(b):
                        chain_ops(off + s + j, ae0[:, (s + j) * F : (s + j + 1) * F],
                                  ta_grp[:, j])
                    prod = prodp.tile([P, 2, 3, F], f32, tag="prod")
                    if first:
                        c_grp = ctiles[s // 2][:]
                    else:
                        c_grp = cchunk[:, s : s + b]
                    ta_b = ta_grp[:, :b].unsqueeze(2).broadcast_to((P, b, 3, F))
                    nc.vector.tensor_tensor(prod[:, :b], c_grp, ta_b, op=op.mult)
                    if off + s == depth - 2:
                        # final pair: accumulate on DVE in SBUF so the PSUM
                        # matmul group closes early and PE drains in parallel
                        tailsum = constp.tile([P, 3, F], f32, tag="tailsum")
                        nc.vector.tensor_tensor(
                            tailsum[:], prod[:, 0], prod[:, 1], op=op.add
                        )
                    else:
                        for j in range(b):
                            nc.tensor.matmul(
                                cacc[:], ident[:],
                                prod[:, j].rearrange("p c f -> p (c f)"),
                                start=(off + s + j == 0),
                                stop=(off + s + j == depth - 3),
                            )
                    s += b

            # out = C_psum + (tailsum + T): the T-fold runs while PE still
            # drains; only one op depends on the final PSUM state
            T_b = T[:].unsqueeze(1).broadcast_to((P, 3, F))
            nc.vector.tensor_tensor(tailsum[:], tailsum[:], T_b, op=op.add)
            out_t = constp.tile([P, 3, F], f32, tag="out")
            nc.vector.tensor_tensor(
                out_t[:], cacc[:].rearrange("p (c f) -> p c f", c=3), tailsum[:],
                op=op.add,
            )
            nc.sync.dma_start(out_d, out_t[:].rearrange("p c f -> p (c f)"))

    nc.compile()
    return nc


def _prep_inputs(color_stroke, alpha, depth):
    """Slice the top `depth` strokes (reversed so stroke 0 = highest index)
    and lay them out per core: alpha [P, depth*F], color [P, depth*3*F]."""
    a_r = alpha[:, N - depth :, 0][:, ::-1]          # (B, depth, P, F)
    c_r = color_stroke[:, N - depth :][:, ::-1]      # (B, depth, 3, P, F)
    ident = np.eye(P, dtype=np.float32)
    in_maps = []
    for b in range(B):
        a_core = np.ascontiguousarray(a_r[b].transpose(1, 0, 2)).reshape(P, depth * F)
        c_core = np.ascontiguousarray(c_r[b].transpose(2, 0, 1, 3)).reshape(
            P, depth * 3 * F
        )
        in_maps.append(
            {"alpha_in": a_core, "color_in": c_core, "ident_in": ident}
        )
    return in_maps


def _reference_numpy(color_stroke, alpha):
    """Exact replication of the oracle (incl. top-k tie-breaking) on host.
    Only used when the depth-cutoff precondition fails (pathological inputs)."""
    stroke_ids = np.arange(1, N + 1, dtype=np.int32).reshape(1, N, 1, 1)
    draw = stroke_ids * (alpha[:, :, 0] > ALPHA_THRESH).astype(np.int32)  # (B,N,W,W)
    draw_t = np.moveaxis(draw, 1, -1)  # (B,W,W,N)
    idx = np.argsort(-draw_t, axis=-1, kind="stable")[..., :K]  # (B,W,W,K)
    idx = np.moveaxis(idx, -1, 1)[:, :, None]  # (B,K,1,W,W)
    alpha_k = np.take_along_axis(alpha, idx, axis=1)  # (B,K,1,W,W)
    color_k = np.take_along_axis(color_stroke, idx, axis=1)  # (B,K,3,W,W)
    canvas = np.ones((B, 3, W, W), dtype=color_stroke.dtype)
    for i in range(K - 1, -1, -1):
        a = alpha_k[:, i]
        canvas = canvas * (1.0 - a) + a * color_k[:, i]
    return canvas


def kernel(color_stroke, alpha):
    color_stroke = np.asarray(color_stroke, dtype=np.float32)
    alpha = np.asarray(alpha, dtype=np.float32)
    assert color_stroke.shape == (B, N, 3, W, W), color_stroke.shape
    assert alpha.shape == (B, N, 1, W, W), alpha.shape

    # Precondition for the depth cutoff: every pixel finds its 10 passing
    # strokes within the top D.  (Exact fixed input needs D* = 30.)
    top_pass = (alpha[:, N - D :, 0] > ALPHA_THRESH).sum(axis=1)
    if top_pass.min() < K:
        return _reference_numpy(color_stroke, alpha)

    from concourse.bass_utils import run_bass_kernel_spmd

    if D not in _nc_cache:
        _nc_cache[D] = _build_nc(D)
    nc = _nc_cache[D]

    in_maps = _prep_inputs(color_stroke, alpha, D)
    res = run_bass_kernel_spmd(nc, in_maps, core_ids=list(range(NCORES)))

    out = np.empty((B, 3, W, W), dtype=np.float32)
    for b in range(B):
        out[b] = res.results[b]["out"].reshape(P, 3, F).transpose(1, 0, 2)
    return out



# revision 3
# speedup vs baseline: 1.7501x; 1.7501x over previous
"""Trainium2 Bass kernel for AttnPainterOil-style top-K stroke compositing.

Problem semantics (per pixel, fully independent):
  draw[n] = (n+1) * (alpha[n] > 0.1); top-K=10 of draw over N=256 strokes
  (descending) == the 10 highest-index strokes with alpha > 0.1.  Gather
  alpha/color at those indices and composite back-to-front over white.

Device formulation (front-to-back, strokes in descending index order):
maintain per-pixel transmittance T (init 1) and a raw pass-count R.  For
stroke s with host-masked alpha ae_s (= a * 1{a > 0.1}, fp32-exact mask
applied on host, shipped as fp16):
  gate m_s = 1{R_s < 10} with R_s = #passing among strokes < s (RAW count,
  independent of gating -- so it batches), ta_s = m_s * b_s * T_quad, where
  b_s are quad-local exclusive-prefix weights b_j = ae_j * prod_{i<j}(1-ae_i)
  (batch-precomputed).  Within a quad the gate mask is a suffix cut, so
  ta_j = m_j * b_j * T is exact and T_new = T - sum_j ta_j.

Key perf facts measured on TRN2 for this kernel family:
  - DVE op = ~150ns dispatch + free_elems cycles @0.96GHz; fp16 with unit
    innermost stride runs 2x (245G elem/s).  Broadcasts on outer dims keep 2x.
  - ACT (scalar engine) is otherwise idle: takes 1-ae, pass bits (Sign),
    small copies at 1 elem/cycle/lane.
  - fp16 everywhere passes easily (8.6e-4 vs 2e-2 tolerance, host-verified).
  - Depth 20 suffices for this input (every pixel's 10th passing stroke is
    within the top 20; checked on host, exact numpy fallback otherwise).
  - PE accumulates ta*c into PSUM via fp16 identity matmuls (4x faster than
    the old fp32 matmuls); GpSimd only does ident DMA + memsets (it shares
    SBUF ports with DVE).

Sharding: pure data parallel, one batch element per NeuronCore (B=8).
"""

import numpy as np

B, N, W, K = 8, 256, 128, 10
ALPHA_THRESH = 0.1
D = 20          # strokes processed from the top (host-verified sufficient)
P = 128         # partitions (pixel rows)
F = 128         # free dim (pixel cols)
NCORES = 8

_nc_cache = {}


def _build_nc(depth):
    import concourse.bass as bass  # noqa: F401
    import concourse.tile as tile
    from concourse import bacc, mybir
    from concourse.vector_clock import ScopedClock

    op = mybir.AluOpType
    act = mybir.ActivationFunctionType
    f16 = mybir.dt.float16
    f32 = mybir.dt.float32

    assert depth % 4 == 0
    NQ = depth // 4          # quads (5)
    NQ1 = 2                  # ungated quads (strokes 0..7)
    NQ2 = NQ - NQ1           # gated quads (strokes 8..depth-1)
    S1, S2 = 4 * NQ1, depth - 4 * NQ1   # strokes per chunk (8, 12)

    class _OneShotTileContext(tile.TileContext):
        """TileContext with a slim exit: the drain alone (it waits on the
        global clock, including output-DMA completion) -- no all-engine
        barriers and no per-semaphore clears.  Safe because every
        run_bass_kernel_spmd call builds and loads a fresh executable."""

        def _drain_and_barrier(self, tick_clock, wait_clock):
            drain_inst = self.nc.sync.drain()
            wait_clock.add_sem_waits(
                drain_inst.ins, ScopedClock({None: tick_clock.global_clock})
            )
            popped = self.nc._tile_sem_poison_stack.pop()
            assert popped is self._sem_poison

    nc = bacc.Bacc("TRN2", target_bir_lowering=False, debug=False)

    ae_d = nc.dram_tensor("ae_in", [P, depth * F], f16, kind="ExternalInput").ap()
    c_d = nc.dram_tensor("color_in", [P, depth * 3 * F], f16, kind="ExternalInput").ap()
    ident_d = nc.dram_tensor("ident_in", [P, P], f16, kind="ExternalInput").ap()
    out_d = nc.dram_tensor("out", [P, 3 * F], f16, kind="ExternalOutput").ap()

    with _OneShotTileContext(nc) as tc:
        with (
            tc.tile_pool(name="const", bufs=1) as constp,
            tc.tile_pool(name="slab", bufs=1) as slabp,
            tc.tile_pool(name="work", bufs=2) as workp,
            tc.tile_pool(name="prodp", bufs=2) as prodp,
            tc.tile_pool(name="psum", bufs=1, space="PSUM") as psump,
        ):
            # ident via SWDGE (gpsimd queue) so it doesn't delay the HWDGE
            # input stream; it's only needed by the first matmul.
            ident = constp.tile([P, P], f16)
            nc.gpsimd.dma_start(ident[:], ident_d)

            T = constp.tile([P, F], f16)
            R = constp.tile([P, F], f16)
            nc.gpsimd.memset(T[:], 1.0)

            # ACT warmup: trigger activation-table loads while input DMA runs
            warm = constp.tile([P, 8], f16)
            nc.gpsimd.memset(warm[:], 0.5)
            wout = constp.tile([P, 8], f16)
            nc.scalar.sign(wout[:], warm[:])
            nc.scalar.activation(wout[:], warm[:], act.Relu, bias=1.0, scale=-1.0)

            # ---- input DMAs ----
            ae1 = slabp.tile([P, S1, F], f16)
            ae2 = slabp.tile([P, S2, F], f16)
            nc.sync.dma_start(
                ae1[:], ae_d[:, : S1 * F].rearrange("p (s f) -> p s f", s=S1)
            )
            nc.sync.dma_start(
                ae2[:], ae_d[:, S1 * F :].rearrange("p (s f) -> p s f", s=S2)
            )
            ctile = slabp.tile([P, depth, 3, F], f16)
            for qi in range(NQ):
                lo = qi * 4 * 3 * F
                nc.sync.dma_start(
                    ctile[:, 4 * qi : 4 * qi + 4],
                    c_d[:, lo : lo + 4 * 3 * F].rearrange(
                        "p (s c f) -> p s c f", s=4, c=3
                    ),
                )

            # ---- ACT: q = 1 - ae (Relu exact for ae in [0,1]); pass = Sign(ae)
            q1 = slabp.tile([P, S1, F], f16)
            q2 = slabp.tile([P, S2, F], f16)
            nc.scalar.activation(q1[:], ae1[:], act.Relu, bias=1.0, scale=-1.0)
            nc.scalar.activation(q2[:], ae2[:], act.Relu, bias=1.0, scale=-1.0)
            pass1 = slabp.tile([P, S1, F], f16)          # strokes 0..7
            pass2 = slabp.tile([P, S2 - 1, F], f16)      # strokes 8..depth-2
            nc.scalar.sign(pass1[:], ae1[:])
            nc.scalar.sign(pass2[:], ae2[:, : S2 - 1])

            # ---- batched b-term precompute (quad-local exclusive prefixes)
            # positions within quad: 0: ae; 1: ae*q0; 2: ae*q01; 3: ae*q012
            def build_b(aet, qt, nq, btile, qq=None):
                aeQ = aet[:].rearrange("p (qd s) f -> p qd s f", s=4)
                qQ = qt[:].rearrange("p (qd s) f -> p qd s f", s=4)
                bQ = btile[:].rearrange("p (qd s) f -> p qd s f", s=4)
                aeP = aet[:].rearrange("p (pr two) f -> p pr two f", two=2)
                qP = qt[:].rearrange("p (pr two) f -> p pr two f", two=2)
                # pos 0 copy on ACT (frees DVE)
                nc.scalar.copy(bQ[:, :, 0], aeQ[:, :, 0])
                # q12 per pair
                q12 = workp.tile([P, 2 * nq, F], f16, tag="q12")
                nc.vector.tensor_tensor(q12[:], qP[:, :, 0], qP[:, :, 1], op=op.mult)
                q12P = q12[:].rearrange("p (qd two) f -> p qd two f", two=2)
                # pos 1 = ae1*q0
                nc.vector.tensor_tensor(
                    bQ[:, :, 1], aeQ[:, :, 1], qQ[:, :, 0], op=op.mult
                )
                # pos 2 = ae2*q12
                nc.vector.tensor_tensor(
                    bQ[:, :, 2], aeQ[:, :, 2], q12P[:, :, 0], op=op.mult
                )
                # pos 3 = (ae3*q2)*q12
                t4 = workp.tile([P, nq, F], f16, tag="t4")
                nc.vector.tensor_tensor(t4[:], aeQ[:, :, 3], qQ[:, :, 2], op=op.mult)
                nc.vector.tensor_tensor(
                    bQ[:, :, 3], t4[:], q12P[:, :, 0], op=op.mult
                )
                if qq is not None:   # quad product for ungated T-update
                    nc.vector.tensor_tensor(
                        qq[:], q12P[:, :, 0], q12P[:, :, 1], op=op.mult
                    )
                return bQ

            b1 = slabp.tile([P, S1, F], f16)
            qq1 = slabp.tile([P, NQ1, F], f16)
            bQ1 = build_b(ae1, q1, NQ1, b1, qq1)

            b2t = slabp.tile([P, S2, F], f16)
            bQ2 = build_b(ae2, q2, NQ2, b2t)

            # ---- gate machinery (batched): pair/quad sums of pass bits,
            # intra-quad partial prefixes for gated quads.
            ps1 = slabp.tile([P, S1 // 2, F], f16)       # pairs 0..3
            p1P = pass1[:].rearrange("p (pr two) f -> p pr two f", two=2)
            nc.vector.tensor_tensor(ps1[:], p1P[:, :, 0], p1P[:, :, 1], op=op.add)
            npair2 = (S2 - 2) // 2                       # pairs 4..8 (strokes 8..17)
            ps2 = slabp.tile([P, npair2, F], f16)
            p2P = pass2[:, : 2 * npair2].rearrange(
                "p (pr two) f -> p pr two f", two=2
            )
            nc.vector.tensor_tensor(ps2[:], p2P[:, :, 0], p2P[:, :, 1], op=op.add)

            qs1 = slabp.tile([P, NQ1, F], f16)
            ps1P = ps1[:].rearrange("p (qd two) f -> p qd two f", two=2)
            nc.vector.tensor_tensor(qs1[:], ps1P[:, :, 0], ps1P[:, :, 1], op=op.add)
            qs2 = slabp.tile([P, NQ2 - 1, F], f16)       # quads 2,3 (for R updates)
            ps2P = ps2[:, : 2 * (NQ2 - 1)].rearrange(
                "p (qd two) f -> p qd two f", two=2
            )
            nc.vector.tensor_tensor(qs2[:], ps2P[:, :, 0], ps2P[:, :, 1], op=op.add)

            # partials for gated quads: j=0: 0; j=1: p0; j=2: p0+p1; j=3: p0+p1+p2
            part = slabp.tile([P, NQ2, 4, F], f16)
            nc.gpsimd.memset(part[:, :, 0], 0.0)
            nc.scalar.copy(part[:, :, 1], pass2[:, 0::4])     # strokes 8,12,16
            nc.scalar.copy(part[:, :, 2], ps2[:, 0::2])       # pairs 4,6,8
            nc.vector.tensor_tensor(
                part[:, :, 3], ps2[:, 0::2], pass2[:, 2::4], op=op.add
            )

            # ---- serial chain over quads ----
            cacc = psump.tile([P, 3 * F], f32)
            for qi in range(NQ):
                gated = qi >= NQ1
                bQ = bQ2 if gated else bQ1
                lq = qi - NQ1 if gated else qi
                T_b = T[:].unsqueeze(1).broadcast_to((P, 4, F))
                ta = workp.tile([P, 4, F], f16, tag="ta")
                if not gated:
                    nc.vector.tensor_tensor(ta[:], bQ[:, lq], T_b, op=op.mult)
                else:
                    tmp = workp.tile([P, 4, F], f16, tag="tmp")
                    R_b = R[:].unsqueeze(1).broadcast_to((P, 4, F))
                    nc.vector.tensor_tensor(tmp[:], part[:, lq], R_b, op=op.add)
                    mb = workp.tile([P, 4, F], f16, tag="mb")
                    nc.vector.scalar_tensor_tensor(
                        mb[:], tmp[:], float(K) - 0.5, bQ[:, lq],
                        op0=op.is_lt, op1=op.mult,
                    )
                    nc.vector.tensor_tensor(ta[:], mb[:], T_b, op=op.mult)

                prod = prodp.tile([P, 4, 3, F], f16, tag="prod")
                ta_b = ta[:].unsqueeze(2).broadcast_to((P, 4, 3, F))
                nc.vector.tensor_tensor(
                    prod[:], ctile[:, 4 * qi : 4 * qi + 4], ta_b, op=op.mult
                )
                for j in range(4):
                    s = 4 * qi + j
                    nc.tensor.matmul(
                        cacc[:], ident[:],
                        prod[:, j].rearrange("p c f -> p (c f)"),
                        start=(s == 0), stop=(s == depth - 1),
                    )

                # T update (after prods consumed ta)
                if not gated:
                    nc.vector.tensor_tensor(T[:], T[:], qq1[:, qi], op=op.mult)
                else:
                    h = workp.tile([P, 2, F], f16, tag="h")
                    nc.vector.tensor_tensor(
                        h[:], ta[:, 0:2], ta[:, 2:4], op=op.add
                    )
                    nc.vector.tensor_tensor(T[:], T[:], h[:, 0], op=op.subtract)
                    nc.vector.tensor_tensor(T[:], T[:], h[:, 1], op=op.subtract)

                # R update (raw pass count at next quad start)
                if qi == NQ1 - 1:
                    nc.vector.tensor_tensor(
                        R[:], qs1[:, 0], qs1[:, 1], op=op.add
                    )
                elif gated and qi < NQ - 1:
                    nc.vector.tensor_tensor(
                        R[:], R[:], qs2[:, lq], op=op.add
                    )

            # out = cacc + T (white background), straight out of PSUM
            outt = constp.tile([P, 3, F], f16)
            nc.vector.tensor_tensor(
                outt[:], cacc[:].rearrange("p (c f) -> p c f", c=3),
                T[:].unsqueeze(1).broadcast_to((P, 3, F)), op=op.add,
            )
            nc.sync.dma_start(out_d, outt[:].rearrange("p c f -> p (c f)"))

    nc.compile()
    return nc


def _prep_inputs(color_stroke, alpha, depth):
    """Slice the top `depth` strokes (reversed so stroke 0 = highest index),
    mask alpha by the fp32-exact threshold on host, convert to fp16 and lay
    out per core: ae [P, depth*F], color [P, depth*3*F]."""
    a_r = alpha[:, N - depth :, 0][:, ::-1]          # (B, depth, P, F)
    ae_host = (a_r * (a_r > ALPHA_THRESH)).astype(np.float16)
    c_r = color_stroke[:, N - depth :][:, ::-1].astype(np.float16)  # (B,depth,3,P,F)
    ident = np.eye(P, dtype=np.float16)
    in_maps = []
    for b in range(B):
        ae_core = np.ascontiguousarray(ae_host[b].transpose(1, 0, 2)).reshape(
            P, depth * F
        )
        c_core = np.ascontiguousarray(c_r[b].transpose(2, 0, 1, 3)).reshape(
            P, depth * 3 * F
        )
        in_maps.append({"ae_in": ae_core, "color_in": c_core, "ident_in": ident})
    return in_maps


def _reference_numpy(color_stroke, alpha):
    """Exact replication of the oracle (incl. top-k tie-breaking) on host.
    Only used when the depth-cutoff precondition fails (pathological inputs)."""
    stroke_ids = np.arange(1, N + 1, dtype=np.int32).reshape(1, N, 1, 1)
    draw = stroke_ids * (alpha[:, :, 0] > ALPHA_THRESH).astype(np.int32)  # (B,N,W,W)
    draw_t = np.moveaxis(draw, 1, -1)  # (B,W,W,N)
    idx = np.argsort(-draw_t, axis=-1, kind="stable")[..., :K]  # (B,W,W,K)
    idx = np.moveaxis(idx, -1, 1)[:, :, None]  # (B,K,1,W,W)
    alpha_k = np.take_along_axis(alpha, idx, axis=1)  # (B,K,1,W,W)
    color_k = np.take_along_axis(color_stroke, idx, axis=1)  # (B,K,3,W,W)
    canvas = np.ones((B, 3, W, W), dtype=color_stroke.dtype)
    for i in range(K - 1, -1, -1):
        a = alpha_k[:, i]
        canvas = canvas * (1.0 - a) + a * color_k[:, i]
    return canvas


def kernel(color_stroke, alpha):
    color_stroke = np.asarray(color_stroke, dtype=np.float32)
    alpha = np.asarray(alpha, dtype=np.float32)
    assert color_stroke.shape == (B, N, 3, W, W), color_stroke.shape
    assert alpha.shape == (B, N, 1, W, W), alpha.shape

    # Precondition for the depth cutoff: every pixel finds its 10 passing
    # strokes within the top D.
    top_pass = (alpha[:, N - D :, 0] > ALPHA_THRESH).sum(axis=1)
    if top_pass.min() < K:
        return _reference_numpy(color_stroke, alpha)

    from concourse.bass_utils import run_bass_kernel_spmd

    if D not in _nc_cache:
        _nc_cache[D] = _build_nc(D)
    nc = _nc_cache[D]

    in_maps = _prep_inputs(color_stroke, alpha, D)
    res = run_bass_kernel_spmd(nc, in_maps, core_ids=list(range(NCORES)))

    out = np.empty((B, 3, W, W), dtype=np.float32)
    for b in range(B):
        out[b] = (
            res.results[b]["out"].reshape(P, 3, F).transpose(1, 0, 2)
            .astype(np.float32)
        )
    return out
